# revision 6
# baseline (speedup 1.0000x reference)
"""Trainium2 Bass kernel for InvertedResidual + CondConv (MoE routing).

Strategy: data-parallel over batch (8 samples/core on 8 cores).
Per sample: 1x1 expand conv (PE, BN1 folded host-side, bias via ones-row),
ReLU evac to padded bf16 layout (ACT), depthwise 3x3 as 9 shifted
scalar_tensor_tensor MACs on DVE (padded-flat trick, parity-fixed via a
shifted copy), ReLU+bias2 evac (ACT), CondConv 1x1 on PE with per-sample
expert-mixed kernel (routing: pooled sums -> tiny matmuls + ACT sigmoid;
k-mix on GPSIMD), residual + bias3 folded in as an extra fp32 matmul block.
"""
import sys
import numpy as np

sys.path.insert(0, "/opt/trn_rl_repo")

import ml_dtypes

EPS = 1e-5
B, C, H, W = 64, 64, 56, 56
HID, E, OUP = 384, 8, 64
HW = H * W                 # 3136
NCORES = 8
BS = B // NCORES           # 8 samples per core
NBLK = HID // 128          # 3 channel blocks
HP, WP = 60, 58            # padded DW layout: data rows 2..57, cols 1..56
FLAT = HP * WP             # 3480
REG0, REGN = 60, 3360      # DW op region [REG0, REG0+REGN)
CH = 448                   # spatial chunk (8 rows); 3136 = 7*448
NCH = HW // CH
bf16 = ml_dtypes.bfloat16

_CACHE = {}


def _build(dw_pe_groups):
    """Build + compile the per-core Bass program.

    dw_pe_groups: set of (sample, block) pairs whose depthwise runs as
    diagonal matmuls on the PE instead of STT on DVE.
    """
    from concourse import bacc, bass, mybir, tile

    f32 = mybir.dt.float32
    bf = mybir.dt.bfloat16
    AF = mybir.ActivationFunctionType
    OP = mybir.AluOpType

    nc = bacc.Bacc("TRN2", target_bir_lowering=False, debug=False)

    xaug_d = nc.dram_tensor("xaug", [BS, C + 1, HW], f32, kind="ExternalInput")
    w1aug_d = nc.dram_tensor("w1aug", [C + 1, HID], f32, kind="ExternalInput")
    wraug_d = nc.dram_tensor("wraug", [C + 1, E], f32, kind="ExternalInput")
    wd9_d = nc.dram_tensor("wd9", [128, NBLK, 9], f32, kind="ExternalInput")
    wd9b_d = nc.dram_tensor("wd9b", [128, NBLK, 9], bf, kind="ExternalInput")
    wearr_d = nc.dram_tensor("wearr", [128, NBLK, E, OUP], bf, kind="ExternalInput")
    resid_d = nc.dram_tensor("resid", [C + 1, OUP], f32, kind="ExternalInput")
    bias2_d = nc.dram_tensor("bias2", [128, NBLK], f32, kind="ExternalInput")
    y_d = nc.dram_tensor("y", [BS, OUP, HW], f32, kind="ExternalOutput")

    with tile.TileContext(nc) as tc:
        with (
            tc.tile_pool(name="const", bufs=1) as cpool,
            tc.tile_pool(name="xp", bufs=2) as xpool,
            tc.tile_pool(name="h1p", bufs=1) as h1pool,
            tc.tile_pool(name="shp", bufs=2) as shpool,
            tc.tile_pool(name="accp", bufs=1) as accpool,
            tc.tile_pool(name="h2p", bufs=1) as h2pool,
            tc.tile_pool(name="op", bufs=2) as opool,
            tc.tile_pool(name="kp", bufs=1) as kpool,
            tc.tile_pool(name="pe_ps", bufs=3, space="PSUM") as pe_ps,
            tc.tile_pool(name="cc_ps", bufs=2, space="PSUM") as cc_ps,
            tc.tile_pool(name="dw_ps", bufs=2, space="PSUM") as dw_ps,
            tc.tile_pool(name="rt_ps", bufs=1, space="PSUM") as rt_ps,
        ):
            # --- constants / weights in SBUF ---
            w1aug = cpool.tile([C + 1, HID], f32, tag="w1aug")
            nc.sync.dma_start(w1aug[:], w1aug_d[:])
            wraug = cpool.tile([C + 1, E], f32, tag="wraug")
            nc.sync.dma_start(wraug[:], wraug_d[:])
            wd9 = cpool.tile([128, NBLK, 9], f32, tag="wd9")
            nc.sync.dma_start(wd9[:], wd9_d[:])
            wd9b = cpool.tile([128, NBLK, 9], bf, tag="wd9b")
            nc.sync.dma_start(wd9b[:], wd9b_d[:])
            wearr = cpool.tile([128, NBLK, E, OUP], bf, tag="wearr")
            nc.sync.dma_start(wearr[:], wearr_d[:])
            resid = cpool.tile([C + 1, OUP], f32, tag="resid")
            nc.sync.dma_start(resid[:], resid_d[:])
            bias2 = cpool.tile([128, NBLK], f32, tag="bias2")
            nc.sync.dma_start(bias2[:], bias2_d[:])
            ones128 = cpool.tile([1, 128], f32, tag="ones128")
            nc.vector.memset(ones128[:], 1.0)
            pooled = cpool.tile([C + 1, BS], f32, tag="pooled")
            nc.vector.memset(pooled[:], 1.0)  # row C stays 1.0 (ones row)
            # diag stationary tiles for PE-mode DW: [128, 9(tap), 128]
            if dw_pe_groups:
                diag = {}
                for b in range(NBLK):
                    if not any(g[1] == b for g in dw_pe_groups):
                        continue
                    dg = cpool.tile([128, 9, 128], bf, tag=f"diag{b}")
                    nc.vector.memset(dg[:], 0.0)
                    diag[b] = dg
                # fill diagonals: copy wd9b column tap -> dg[p, tap, p]
                # (per-partition scalar write via iota trick is complex; instead
                #  ship diagonals from host)
            # host-shipped diagonals (simpler): dgm[p, b, tap, 128]
            dgm_d = None
            if dw_pe_groups:
                dgm_d = nc.dram_tensor(
                    "dgm", [128, NBLK, 9, 128], bf, kind="ExternalInput"
                )
                dgm = cpool.tile([128, NBLK, 9, 128], bf, tag="dgm")
                nc.sync.dma_start(dgm[:], dgm_d[:])

            # persistent padded h1 tiles (pads must stay zero): [blk][parity]
            h1t = [
                [h1pool.tile([128, FLAT], bf, tag=f"h1_{b}_{p}", name=f"h1_{b}_{p}") for p in range(2)]
                for b in range(NBLK)
            ]
            for b in range(NBLK):
                for p in range(2):
                    nc.vector.memset(h1t[b][p][:], 0.0)
            acct = [accpool.tile([128, FLAT], bf, tag=f"acc_{i}", name=f"acc_{i}") for i in range(2)]
            h2t = [
                [h2pool.tile([128, HW], bf, tag=f"h2_{b}_{p}", name=f"h2_{b}_{p}") for p in range(2)]
                for b in range(NBLK)
            ]
            kt = [
                [kpool.tile([128, NBLK, OUP], bf, tag=f"k_{i}_{p}", name=f"k_{i}_{p}") for i in range(2)]
                for p in range(2)
            ]
            scratch = cpool.tile([C, HW], bf, tag="scratch")

            for s in range(BS):
                par = s % 2
                xt = xpool.tile([C + 1, HW], f32, tag="xt")
                nc.sync.dma_start(xt[:], xaug_d[s])

                # ---- routing ----
                nc.vector.tensor_reduce(
                    pooled[0:C, s : s + 1],
                    xt[0:C, :],
                    mybir.AxisListType.X,
                    OP.add,
                )
                ps_rt = rt_ps.tile([1, E], f32, tag="ps_rt")
                nc.tensor.matmul(
                    ps_rt[:], pooled[:, s : s + 1], wraug[:], start=True, stop=True
                )
                sig = xpool.tile([1, E], f32, tag="sig")
                nc.scalar.activation(sig[:], ps_rt[:], AF.Sigmoid)
                ps_r = rt_ps.tile([128, E], f32, tag="ps_r")
                nc.tensor.matmul(ps_r[:], ones128[:], sig[:], start=True, stop=True)
                rsb = xpool.tile([128, E], f32, tag="rsb")
                nc.scalar.copy(rsb[:], ps_r[:])
                # ---- k-mix on GPSIMD: k[c, b, o] = sum_e r_e * we[c, b, e, o]
                ka, kb = kt[par]
                nc.vector.tensor_scalar(
                    ka[:], wearr[:, :, 0, :], rsb[:, 0:1], None, OP.mult
                )
                cur, nxt = ka, kb
                for e in range(1, E):
                    nc.vector.scalar_tensor_tensor(
                        nxt[:], wearr[:, :, e, :], rsb[:, e : e + 1], cur[:],
                        OP.mult, OP.add,
                    )
                    cur, nxt = nxt, cur
                kfin = cur  # after 7 STT: ends in kb... (E-1 odd -> kb)

                # ---- expand conv + ReLU evac into padded h1 ----
                for b in range(NBLK):
                    hv = h1t[b][par].rearrange("p (h w) -> p h w", h=HP, w=WP)
                    for j in range(NCH):
                        ps = pe_ps.tile([128, CH], f32, tag="ps_e")
                        nc.tensor.matmul(
                            ps[:],
                            w1aug[:, b * 128 : (b + 1) * 128],
                            xt[:, j * CH : (j + 1) * CH],
                            start=True,
                            stop=True,
                        )
                        nc.scalar.activation(
                            hv[:, 2 + 8 * j : 10 + 8 * j, 1:57],
                            ps.rearrange("p (h w) -> p h w", h=8, w=56)[:],
                            AF.Relu,
                        )

                # ---- depthwise 3x3 ----
                for b in range(NBLK):
                    h1 = h1t[b][par]
                    if (s, b) in dw_pe_groups:
                        # PE path: 9 diag matmuls accumulating in PSUM per chunk
                        hv = h1.rearrange("p (h w) -> p h w", h=HP, w=WP)
                        for j in range(NCH):
                            ps = dw_ps.tile([128, CH], f32, tag="ps_dw")
                            first = True
                            for dh in (-1, 0, 1):
                                for dw_ in (-1, 0, 1):
                                    tap = (dh + 1) * 3 + (dw_ + 1)
                                    nc.tensor.matmul(
                                        ps.rearrange("p (h w) -> p h w", h=8, w=56)[:],
                                        dgm[:, b, tap, :],
                                        hv[
                                            :,
                                            2 + 8 * j + dh : 10 + 8 * j + dh,
                                            1 + dw_ : 57 + dw_,
                                        ],
                                        start=first,
                                        stop=(tap == 8),
                                    )
                                    first = False
                            nc.scalar.activation(
                                h2t[b][par][:, j * CH : (j + 1) * CH],
                                ps[:],
                                AF.Relu,
                                bias=bias2[:, b : b + 1],
                            )
                    else:
                        # DVE path: shifted copy + 9 STT MACs (all 2x-aligned)
                        sh = shpool.tile([128, FLAT], bf, tag="sh")
                        nc.gpsimd.tensor_copy(sh[:, 0 : FLAT - 1], h1[:, 1:FLAT])
                        aa, ab = acct
                        first = True
                        cur_acc = None
                        for dh in (-1, 0, 1):
                            for dw_ in (-1, 0, 1):
                                tap = (dh + 1) * 3 + (dw_ + 1)
                                d = WP * dh + dw_
                                wtap = wd9[:, b, tap : tap + 1]
                                if d % 2 == 0:
                                    opnd = h1[:, REG0 + d : REG0 + d + REGN]
                                else:
                                    opnd = sh[:, REG0 + d - 1 : REG0 + d - 1 + REGN]
                                if first:
                                    nc.vector.tensor_scalar(
                                        aa[:, REG0 : REG0 + REGN],
                                        opnd, wtap, None, OP.mult,
                                    )
                                    cur_acc, nxt_acc = aa, ab
                                    first = False
                                else:
                                    nc.vector.scalar_tensor_tensor(
                                        nxt_acc[:, REG0 : REG0 + REGN],
                                        opnd, wtap,
                                        cur_acc[:, REG0 : REG0 + REGN],
                                        OP.mult, OP.add,
                                    )
                                    cur_acc, nxt_acc = nxt_acc, cur_acc
                        av = cur_acc.rearrange("p (h w) -> p h w", h=HP, w=WP)
                        nc.scalar.activation(
                            h2t[b][par].rearrange("p (h w) -> p h w", h=H, w=W)[:],
                            av[:, 2:58, 1:57],
                            AF.Relu,
                            bias=bias2[:, b : b + 1],
                        )

                # ---- condconv + residual ----
                ot = opool.tile([OUP, HW], f32, tag="ot")
                for j in range(NCH):
                    ps = cc_ps.tile([OUP, CH], f32, tag="ps_c")
                    for b in range(NBLK):
                        nc.tensor.matmul(
                            ps[:],
                            kfin[:, b, :],
                            h2t[b][par][:, j * CH : (j + 1) * CH],
                            start=(b == 0),
                            stop=False,
                        )
                    nc.tensor.matmul(
                        ps[:],
                        resid[:],
                        xt[:, j * CH : (j + 1) * CH],
                        start=False,
                        stop=True,
                    )
                    nc.scalar.copy(ot[:, j * CH : (j + 1) * CH], ps[:])
                nc.sync.dma_start(y_d[s], ot[:])

    nc.compile()
    return nc


def _prep(inputs):
    x = np.ascontiguousarray(inputs["x"].reshape(B, C, HW), dtype=np.float32)
    xaug = np.empty((B, C + 1, HW), np.float32)
    xaug[:, :C, :] = x
    xaug[:, C, :] = 1.0

    def bnfold(g, b, m, v):
        s = np.asarray(g, np.float32) / np.sqrt(np.asarray(v, np.float32) + EPS)
        return s, np.asarray(b, np.float32) - np.asarray(m, np.float32) * s

    s1, b1 = bnfold(inputs["g1"], inputs["b1"], inputs["m1"], inputs["v1"])
    s2, b2 = bnfold(inputs["g2"], inputs["b2"], inputs["m2"], inputs["v2"])
    s3, b3 = bnfold(inputs["g3"], inputs["b3"], inputs["m3"], inputs["v3"])
    w1 = np.asarray(inputs["w1"], np.float32)
    wd = np.asarray(inputs["wd"], np.float32)
    we = np.asarray(inputs["we"], np.float32)
    wr = np.asarray(inputs["wr"], np.float32)
    br = np.asarray(inputs["br"], np.float32)

    w1aug = np.zeros((C + 1, HID), np.float32)
    w1aug[:C, :] = (w1 * s1[:, None]).T
    w1aug[C, :] = b1
    wraug = np.zeros((C + 1, E), np.float32)
    wraug[:C, :] = wr.T / HW
    wraug[C, :] = br
    wd9 = (wd[:, 0, :, :] * s2[:, None, None]).reshape(HID, 9)
    wd9r = np.ascontiguousarray(wd9.reshape(NBLK, 128, 9).transpose(1, 0, 2))
    we_s = we * s3[None, :, None]                      # [E, OUP, HID]
    wearr = np.ascontiguousarray(
        np.transpose(we_s, (2, 0, 1)).reshape(NBLK, 128, E, OUP).transpose(1, 0, 2, 3)
    ).astype(bf16)
    # wearr[p, b, e, o] = we_s[e, o, 128*b + p]
    wearr = np.ascontiguousarray(
        np.transpose(we_s, (2, 0, 1))  # [c, e, o]
        .reshape(NBLK, 128, E, OUP)    # [b, p, e, o]
        .transpose(1, 0, 2, 3)         # [p, b, e, o]
    ).astype(bf16)
    resid = np.zeros((C + 1, OUP), np.float32)
    resid[:C, :] = np.eye(C, dtype=np.float32)
    resid[C, :] = b3
    # diag matrices for PE-mode DW: dgm[p, b, tap, q] = wd9[128b+p, tap] if p==q
    dgm = np.zeros((128, NBLK, 9, 128), np.float32)
    idx = np.arange(128)
    for b in range(NBLK):
        for t in range(9):
            dgm[idx, b, t, idx] = wd9[b * 128 + idx, t]
    dgm = dgm.astype(bf16)
    shared = dict(
        w1aug=w1aug, wraug=wraug, wd9=wd9r, wd9b=wd9r.astype(bf16), wearr=wearr,
        resid=resid, bias2=np.ascontiguousarray(b2.reshape(NBLK, 128).T), dgm=dgm,
    )
    return xaug, shared


DW_PE_GROUPS = frozenset()


def kernel(**inputs):
    from concourse.bass_utils import run_bass_kernel_spmd

    key = ("k", DW_PE_GROUPS)
    if key not in _CACHE:
        _CACHE[key] = _build(DW_PE_GROUPS)
    nc = _CACHE[key]

    xaug, shared = _prep(inputs)
    if not DW_PE_GROUPS:
        shared = {k: v for k, v in shared.items() if k != "dgm"}
    in_maps = [
        dict(shared, xaug=np.ascontiguousarray(xaug[c * BS : (c + 1) * BS]))
        for c in range(NCORES)
    ]
    res = run_bass_kernel_spmd(nc, in_maps, core_ids=list(range(NCORES)))
    y = np.concatenate([r["y"] for r in res.results], axis=0)
    return y.reshape(B, OUP, H, W).astype(np.float32)


# revision 8
# speedup vs baseline: 1.6025x; 1.6025x over previous
"""Trainium2 Bass kernel for InvertedResidual + CondConv (MoE routing).

Strategy: data-parallel over batch (8 samples/core on 8 cores).
Per sample: 1x1 expand conv (PE, BN1 folded host-side, bias via ones-row),
ReLU evac to padded bf16 layout (ACT), depthwise 3x3 as 9 shifted
scalar_tensor_tensor MACs on DVE (padded-flat trick, parity-fixed via a
shifted copy), ReLU+bias2 evac (ACT), CondConv 1x1 on PE with per-sample
expert-mixed kernel (routing: pooled sums -> tiny matmuls + ACT sigmoid;
k-mix on GPSIMD), residual + bias3 folded in as an extra fp32 matmul block.
"""
import sys
import numpy as np

sys.path.insert(0, "/opt/trn_rl_repo")

import ml_dtypes

EPS = 1e-5
B, C, H, W = 64, 64, 56, 56
HID, E, OUP = 384, 8, 64
HW = H * W                 # 3136
NCORES = 8
BS = B // NCORES           # 8 samples per core
NBLK = HID // 128          # 3 channel blocks
HP, WP = 60, 58            # padded DW layout: data rows 2..57, cols 1..56
FLAT = HP * WP             # 3480
REG0, REGN = 60, 3360      # DW op region [REG0, REG0+REGN)
CH = 448                   # spatial chunk (8 rows); 3136 = 7*448
NCH = HW // CH
bf16 = ml_dtypes.bfloat16

_CACHE = {}


def _build(dw_pe_groups):
    """Build + compile the per-core Bass program.

    dw_pe_groups: set of (sample, block) pairs whose depthwise runs as
    diagonal matmuls on the PE instead of STT on DVE.
    """
    from concourse import bacc, bass, mybir, tile

    f32 = mybir.dt.float32
    bf = mybir.dt.bfloat16
    AF = mybir.ActivationFunctionType
    OP = mybir.AluOpType

    nc = bacc.Bacc("TRN2", target_bir_lowering=False, debug=False)

    xaug_d = nc.dram_tensor("xaug", [BS, C + 1, HW], f32, kind="ExternalInput")
    w1aug_d = nc.dram_tensor("w1aug", [C + 1, HID], f32, kind="ExternalInput")
    wraug_d = nc.dram_tensor("wraug", [C + 1, E], f32, kind="ExternalInput")
    wd9_d = nc.dram_tensor("wd9", [128, NBLK, 9], f32, kind="ExternalInput")
    wd9b_d = nc.dram_tensor("wd9b", [128, NBLK, 9], bf, kind="ExternalInput")
    wearr_d = nc.dram_tensor("wearr", [128, NBLK, E, OUP], bf, kind="ExternalInput")
    resid_d = nc.dram_tensor("resid", [C + 1, OUP], f32, kind="ExternalInput")
    bias2_d = nc.dram_tensor("bias2", [128, NBLK], f32, kind="ExternalInput")
    y_d = nc.dram_tensor("y", [BS, OUP, HW], f32, kind="ExternalOutput")

    with tile.TileContext(nc) as tc:
        with (
            tc.tile_pool(name="const", bufs=1) as cpool,
            tc.tile_pool(name="xp", bufs=2) as xpool,
            tc.tile_pool(name="h1p", bufs=1) as h1pool,
            tc.tile_pool(name="shp", bufs=2) as shpool,
            tc.tile_pool(name="accp", bufs=1) as accpool,
            tc.tile_pool(name="h2p", bufs=1) as h2pool,
            tc.tile_pool(name="op", bufs=2) as opool,
            tc.tile_pool(name="kp", bufs=1) as kpool,
            tc.tile_pool(name="pe_ps", bufs=2, space="PSUM") as pe_ps,
            tc.tile_pool(name="cc_ps", bufs=2, space="PSUM") as cc_ps,
            tc.tile_pool(name="dw_ps", bufs=2, space="PSUM") as dw_ps,
            tc.tile_pool(name="rt_ps", bufs=1, space="PSUM") as rt_ps,
        ):
            # --- constants / weights in SBUF ---
            w1aug = cpool.tile([C + 1, HID], f32, tag="w1aug")
            nc.sync.dma_start(w1aug[:], w1aug_d[:])
            wraug = cpool.tile([C + 1, E], f32, tag="wraug")
            nc.sync.dma_start(wraug[:], wraug_d[:])
            wd9 = cpool.tile([128, NBLK, 9], f32, tag="wd9")
            nc.sync.dma_start(wd9[:], wd9_d[:])
            wd9b = cpool.tile([128, NBLK, 9], bf, tag="wd9b")
            nc.sync.dma_start(wd9b[:], wd9b_d[:])
            wearr = cpool.tile([128, NBLK, E, OUP], bf, tag="wearr")
            nc.sync.dma_start(wearr[:], wearr_d[:])
            resid = cpool.tile([C + 1, OUP], f32, tag="resid")
            nc.sync.dma_start(resid[:], resid_d[:])
            bias2 = cpool.tile([128, NBLK], f32, tag="bias2")
            nc.sync.dma_start(bias2[:], bias2_d[:])
            ones128 = cpool.tile([1, 128], f32, tag="ones128")
            nc.vector.memset(ones128[:], 1.0)
            pooled = cpool.tile([C + 1, BS], f32, tag="pooled")
            nc.vector.memset(pooled[:], 1.0)  # row C stays 1.0 (ones row)
            # host-shipped diagonal stationaries for PE-mode DW
            dgm_d = None
            if dw_pe_groups:
                dgm_d = nc.dram_tensor(
                    "dgm", [128, NBLK, 9, 128], bf, kind="ExternalInput"
                )
                dgm = cpool.tile([128, NBLK, 9, 128], bf, tag="dgm")
                nc.sync.dma_start(dgm[:], dgm_d[:])

            # persistent padded h1 tiles (pads must stay zero): [blk][parity]
            h1t = [
                [h1pool.tile([128, FLAT], bf, tag=f"h1_{b}_{p}", name=f"h1_{b}_{p}") for p in range(2)]
                for b in range(NBLK)
            ]
            for b in range(NBLK):
                for p in range(2):
                    nc.vector.memset(h1t[b][p][:], 0.0)
            acct = [accpool.tile([128, FLAT], bf, tag=f"acc_{i}", name=f"acc_{i}") for i in range(2)]
            h2t = [
                [h2pool.tile([128, HW], bf, tag=f"h2_{b}_{p}", name=f"h2_{b}_{p}") for p in range(2)]
                for b in range(NBLK)
            ]
            kt = [
                [kpool.tile([128, NBLK, OUP], bf, tag=f"k_{i}_{p}", name=f"k_{i}_{p}") for i in range(2)]
                for p in range(2)
            ]
            scratch = cpool.tile([C, HW], bf, tag="scratch")

            for s in range(BS):
                par = s % 2
                xt = xpool.tile([C + 1, HW], f32, tag="xt")
                nc.sync.dma_start(xt[:], xaug_d[s])

                # ---- routing ----
                nc.vector.tensor_reduce(
                    pooled[0:C, s : s + 1],
                    xt[0:C, :],
                    mybir.AxisListType.X,
                    OP.add,
                )
                ps_rt = rt_ps.tile([1, E], f32, tag="ps_rt")
                nc.tensor.matmul(
                    ps_rt[:], pooled[:, s : s + 1], wraug[:], start=True, stop=True
                )
                sig = xpool.tile([1, E], f32, tag="sig")
                nc.scalar.activation(sig[:], ps_rt[:], AF.Sigmoid)
                ps_r = rt_ps.tile([128, E], f32, tag="ps_r")
                nc.tensor.matmul(ps_r[:], ones128[:], sig[:], start=True, stop=True)
                rsb = xpool.tile([128, E], f32, tag="rsb")
                nc.scalar.copy(rsb[:], ps_r[:])
                # ---- k-mix on GPSIMD: k[c, b, o] = sum_e r_e * we[c, b, e, o]
                ka, kb = kt[par]
                nc.vector.tensor_scalar(
                    ka[:], wearr[:, :, 0, :], rsb[:, 0:1], None, OP.mult
                )
                cur, nxt = ka, kb
                for e in range(1, E):
                    nc.vector.scalar_tensor_tensor(
                        nxt[:], wearr[:, :, e, :], rsb[:, e : e + 1], cur[:],
                        OP.mult, OP.add,
                    )
                    cur, nxt = nxt, cur
                kfin = cur  # after 7 STT: ends in kb... (E-1 odd -> kb)

                # ---- expand conv + ReLU evac into padded h1 ----
                for b in range(NBLK):
                    hv = h1t[b][par].rearrange("p (h w) -> p h w", h=HP, w=WP)
                    for j in range(NCH):
                        ps = pe_ps.tile([128, CH], f32, tag="ps_e")
                        nc.tensor.matmul(
                            ps[:],
                            w1aug[:, b * 128 : (b + 1) * 128],
                            xt[:, j * CH : (j + 1) * CH],
                            start=True,
                            stop=True,
                        )
                        nc.scalar.activation(
                            hv[:, 2 + 8 * j : 10 + 8 * j, 1:57],
                            ps.rearrange("p (h w) -> p h w", h=8, w=56)[:],
                            AF.Relu,
                        )

                # ---- depthwise 3x3 ----
                for b in range(NBLK):
                    h1 = h1t[b][par]
                    if (s, b) in dw_pe_groups:
                        # PE path: 9 diag matmuls accumulating in PSUM per chunk
                        hv = h1.rearrange("p (h w) -> p h w", h=HP, w=WP)
                        for j in range(NCH):
                            ps = dw_ps.tile([128, CH], f32, tag="ps_dw")
                            first = True
                            for dh in (-1, 0, 1):
                                for dw_ in (-1, 0, 1):
                                    tap = (dh + 1) * 3 + (dw_ + 1)
                                    nc.tensor.matmul(
                                        ps.rearrange("p (h w) -> p h w", h=8, w=56)[:],
                                        dgm[:, b, tap, :],
                                        hv[
                                            :,
                                            2 + 8 * j + dh : 10 + 8 * j + dh,
                                            1 + dw_ : 57 + dw_,
                                        ],
                                        start=first,
                                        stop=(tap == 8),
                                    )
                                    first = False
                            nc.scalar.activation(
                                h2t[b][par][:, j * CH : (j + 1) * CH],
                                ps[:],
                                AF.Relu,
                                bias=bias2[:, b : b + 1],
                            )
                    else:
                        # DVE path: shifted copy + 9 STT MACs (all 2x-aligned)
                        sh = shpool.tile([128, FLAT], bf, tag="sh")
                        nc.gpsimd.tensor_copy(sh[:, 0 : FLAT - 1], h1[:, 1:FLAT])
                        aa, ab = acct
                        first = True
                        cur_acc = None
                        for dh in (-1, 0, 1):
                            for dw_ in (-1, 0, 1):
                                tap = (dh + 1) * 3 + (dw_ + 1)
                                d = WP * dh + dw_
                                wtap = wd9[:, b, tap : tap + 1]
                                if d % 2 == 0:
                                    opnd = h1[:, REG0 + d : REG0 + d + REGN]
                                else:
                                    opnd = sh[:, REG0 + d - 1 : REG0 + d - 1 + REGN]
                                if first:
                                    nc.vector.tensor_scalar(
                                        aa[:, REG0 : REG0 + REGN],
                                        opnd, wtap, None, OP.mult,
                                    )
                                    cur_acc, nxt_acc = aa, ab
                                    first = False
                                else:
                                    nc.vector.scalar_tensor_tensor(
                                        nxt_acc[:, REG0 : REG0 + REGN],
                                        opnd, wtap,
                                        cur_acc[:, REG0 : REG0 + REGN],
                                        OP.mult, OP.add,
                                    )
                                    cur_acc, nxt_acc = nxt_acc, cur_acc
                        av = cur_acc.rearrange("p (h w) -> p h w", h=HP, w=WP)
                        nc.scalar.activation(
                            h2t[b][par].rearrange("p (h w) -> p h w", h=H, w=W)[:],
                            av[:, 2:58, 1:57],
                            AF.Relu,
                            bias=bias2[:, b : b + 1],
                        )

                # ---- condconv + residual ----
                ot = opool.tile([OUP, HW], f32, tag="ot")
                for j in range(NCH):
                    ps = cc_ps.tile([OUP, CH], f32, tag="ps_c")
                    for b in range(NBLK):
                        nc.tensor.matmul(
                            ps[:],
                            kfin[:, b, :],
                            h2t[b][par][:, j * CH : (j + 1) * CH],
                            start=(b == 0),
                            stop=False,
                        )
                    nc.tensor.matmul(
                        ps[:],
                        resid[:],
                        xt[:, j * CH : (j + 1) * CH],
                        start=False,
                        stop=True,
                    )
                    nc.scalar.copy(ot[:, j * CH : (j + 1) * CH], ps[:])
                nc.sync.dma_start(y_d[s], ot[:])

    nc.compile()
    return nc


def _prep(inputs):
    x = np.ascontiguousarray(inputs["x"].reshape(B, C, HW), dtype=np.float32)
    xaug = np.empty((B, C + 1, HW), np.float32)
    xaug[:, :C, :] = x
    xaug[:, C, :] = 1.0

    def bnfold(g, b, m, v):
        s = np.asarray(g, np.float32) / np.sqrt(np.asarray(v, np.float32) + EPS)
        return s, np.asarray(b, np.float32) - np.asarray(m, np.float32) * s

    s1, b1 = bnfold(inputs["g1"], inputs["b1"], inputs["m1"], inputs["v1"])
    s2, b2 = bnfold(inputs["g2"], inputs["b2"], inputs["m2"], inputs["v2"])
    s3, b3 = bnfold(inputs["g3"], inputs["b3"], inputs["m3"], inputs["v3"])
    w1 = np.asarray(inputs["w1"], np.float32)
    wd = np.asarray(inputs["wd"], np.float32)
    we = np.asarray(inputs["we"], np.float32)
    wr = np.asarray(inputs["wr"], np.float32)
    br = np.asarray(inputs["br"], np.float32)

    w1aug = np.zeros((C + 1, HID), np.float32)
    w1aug[:C, :] = (w1 * s1[:, None]).T
    w1aug[C, :] = b1
    wraug = np.zeros((C + 1, E), np.float32)
    wraug[:C, :] = wr.T / HW
    wraug[C, :] = br
    wd9 = (wd[:, 0, :, :] * s2[:, None, None]).reshape(HID, 9)
    wd9r = np.ascontiguousarray(wd9.reshape(NBLK, 128, 9).transpose(1, 0, 2))
    we_s = we * s3[None, :, None]                      # [E, OUP, HID]
    wearr = np.ascontiguousarray(
        np.transpose(we_s, (2, 0, 1)).reshape(NBLK, 128, E, OUP).transpose(1, 0, 2, 3)
    ).astype(bf16)
    # wearr[p, b, e, o] = we_s[e, o, 128*b + p]
    wearr = np.ascontiguousarray(
        np.transpose(we_s, (2, 0, 1))  # [c, e, o]
        .reshape(NBLK, 128, E, OUP)    # [b, p, e, o]
        .transpose(1, 0, 2, 3)         # [p, b, e, o]
    ).astype(bf16)
    resid = np.zeros((C + 1, OUP), np.float32)
    resid[:C, :] = np.eye(C, dtype=np.float32)
    resid[C, :] = b3
    # diag matrices for PE-mode DW: dgm[p, b, tap, q] = wd9[128b+p, tap] if p==q
    dgm = np.zeros((128, NBLK, 9, 128), np.float32)
    idx = np.arange(128)
    for b in range(NBLK):
        for t in range(9):
            dgm[idx, b, t, idx] = wd9[b * 128 + idx, t]
    dgm = dgm.astype(bf16)
    shared = dict(
        w1aug=w1aug, wraug=wraug, wd9=wd9r, wd9b=wd9r.astype(bf16), wearr=wearr,
        resid=resid, bias2=np.ascontiguousarray(b2.reshape(NBLK, 128).T), dgm=dgm,
    )
    return xaug, shared


DW_PE_GROUPS = frozenset({(s, 0) for s in range(BS)} | {(s, 1) for s in range(0, BS, 2)})


def kernel(**inputs):
    from concourse.bass_utils import run_bass_kernel_spmd

    key = ("k", DW_PE_GROUPS)
    if key not in _CACHE:
        _CACHE[key] = _build(DW_PE_GROUPS)
    nc = _CACHE[key]

    xaug, shared = _prep(inputs)
    if not DW_PE_GROUPS:
        shared = {k: v for k, v in shared.items() if k != "dgm"}
    in_maps = [
        dict(shared, xaug=np.ascontiguousarray(xaug[c * BS : (c + 1) * BS]))
        for c in range(NCORES)
    ]
    res = run_bass_kernel_spmd(nc, in_maps, core_ids=list(range(NCORES)))
    y = np.concatenate([r["y"] for r in res.results], axis=0)
    return y.reshape(B, OUP, H, W).astype(np.float32)


# revision 9
# speedup vs baseline: 2.3515x; 1.4674x over previous
"""Trainium2 Bass kernel for InvertedResidual + CondConv (MoE routing).

Strategy: data-parallel over batch (8 samples/core on 8 cores).
Per sample: 1x1 expand conv (PE, BN1 folded host-side, bias via ones-row),
ReLU evac to padded bf16 layout (ACT), depthwise 3x3 as 9 shifted
scalar_tensor_tensor MACs on DVE (padded-flat trick, parity-fixed via a
shifted copy), ReLU+bias2 evac (ACT), CondConv 1x1 on PE with per-sample
expert-mixed kernel (routing: pooled sums -> tiny matmuls + ACT sigmoid;
k-mix on GPSIMD), residual + bias3 folded in as an extra fp32 matmul block.
"""
import sys
import numpy as np

sys.path.insert(0, "/opt/trn_rl_repo")

import ml_dtypes

EPS = 1e-5
B, C, H, W = 64, 64, 56, 56
HID, E, OUP = 384, 8, 64
HW = H * W                 # 3136
NCORES = 8
BS = B // NCORES           # 8 samples per core
NBLK = HID // 128          # 3 channel blocks
HP, WP = 60, 58            # padded DW layout: data rows 2..57, cols 1..56
FLAT = HP * WP             # 3480
REG0, REGN = 60, 3360      # DW op region [REG0, REG0+REGN)
CH = 448                   # spatial chunk (8 rows); 3136 = 7*448
NCH = HW // CH
bf16 = ml_dtypes.bfloat16

_CACHE = {}


def _build(dw_pe_groups):
    """Build + compile the per-core Bass program.

    dw_pe_groups: set of (sample, block) pairs whose depthwise runs as
    diagonal matmuls on the PE instead of STT on DVE.
    """
    from concourse import bacc, bass, mybir, tile

    f32 = mybir.dt.float32
    bf = mybir.dt.bfloat16
    AF = mybir.ActivationFunctionType
    OP = mybir.AluOpType

    nc = bacc.Bacc("TRN2", target_bir_lowering=False, debug=False)

    xaug_d = nc.dram_tensor("xaug", [BS, C + 1, HW], f32, kind="ExternalInput")
    w1aug_d = nc.dram_tensor("w1aug", [C + 1, HID], f32, kind="ExternalInput")
    wraug_d = nc.dram_tensor("wraug", [C + 1, E], f32, kind="ExternalInput")
    wd9_d = nc.dram_tensor("wd9", [128, NBLK, 9], f32, kind="ExternalInput")
    wd9b_d = nc.dram_tensor("wd9b", [128, NBLK, 9], bf, kind="ExternalInput")
    wearr_d = nc.dram_tensor("wearr", [128, NBLK, E, OUP], bf, kind="ExternalInput")
    resid_d = nc.dram_tensor("resid", [C + 1, OUP], f32, kind="ExternalInput")
    bias2_d = nc.dram_tensor("bias2", [128, NBLK], f32, kind="ExternalInput")
    y_d = nc.dram_tensor("y", [BS, OUP, HW], f32, kind="ExternalOutput")

    with tile.TileContext(nc) as tc:
        with (
            tc.tile_pool(name="const", bufs=1) as cpool,
            tc.tile_pool(name="xp", bufs=2) as xpool,
            tc.tile_pool(name="h1p", bufs=1) as h1pool,
            tc.tile_pool(name="shp", bufs=2) as shpool,
            tc.tile_pool(name="accp", bufs=1) as accpool,
            tc.tile_pool(name="h2p", bufs=1) as h2pool,
            tc.tile_pool(name="op", bufs=2) as opool,
            tc.tile_pool(name="kp", bufs=1) as kpool,
            tc.tile_pool(name="pe_ps", bufs=2, space="PSUM") as pe_ps,
            tc.tile_pool(name="cc_ps", bufs=2, space="PSUM") as cc_ps,
            tc.tile_pool(name="dw_ps", bufs=2, space="PSUM") as dw_ps,
            tc.tile_pool(name="rt_ps", bufs=1, space="PSUM") as rt_ps,
        ):
            # --- constants / weights in SBUF ---
            w1aug = cpool.tile([C + 1, HID], f32, tag="w1aug")
            nc.sync.dma_start(w1aug[:], w1aug_d[:])
            wraug = cpool.tile([C + 1, E], f32, tag="wraug")
            nc.sync.dma_start(wraug[:], wraug_d[:])
            wd9 = cpool.tile([128, NBLK, 9], f32, tag="wd9")
            nc.sync.dma_start(wd9[:], wd9_d[:])
            wd9b = cpool.tile([128, NBLK, 9], bf, tag="wd9b")
            nc.sync.dma_start(wd9b[:], wd9b_d[:])
            wearr = cpool.tile([128, NBLK, E, OUP], bf, tag="wearr")
            nc.sync.dma_start(wearr[:], wearr_d[:])
            resid = cpool.tile([C + 1, OUP], f32, tag="resid")
            nc.sync.dma_start(resid[:], resid_d[:])
            bias2 = cpool.tile([128, NBLK], f32, tag="bias2")
            nc.sync.dma_start(bias2[:], bias2_d[:])
            ones128 = cpool.tile([1, 128], f32, tag="ones128")
            nc.vector.memset(ones128[:], 1.0)
            pooled = cpool.tile([C + 1, BS], f32, tag="pooled")
            nc.vector.memset(pooled[:], 1.0)  # row C stays 1.0 (ones row)
            # host-shipped diagonal stationaries for PE-mode DW
            dgm_d = None
            if dw_pe_groups:
                dgm_d = nc.dram_tensor(
                    "dgm", [128, NBLK, 9, 128], bf, kind="ExternalInput"
                )
                dgm = cpool.tile([128, NBLK, 9, 128], bf, tag="dgm")
                nc.sync.dma_start(dgm[:], dgm_d[:])

            # persistent padded h1 tiles (pads must stay zero): [blk][parity]
            h1t = [
                [h1pool.tile([128, FLAT], bf, tag=f"h1_{b}_{p}", name=f"h1_{b}_{p}") for p in range(2)]
                for b in range(NBLK)
            ]
            for b in range(NBLK):
                for p in range(2):
                    nc.vector.memset(h1t[b][p][:], 0.0)
            acct = [accpool.tile([128, FLAT], bf, tag=f"acc_{i}", name=f"acc_{i}") for i in range(2)]
            h2t = [
                [h2pool.tile([128, HW], bf, tag=f"h2_{b}_{p}", name=f"h2_{b}_{p}") for p in range(2)]
                for b in range(NBLK)
            ]
            kt = [
                [kpool.tile([128, NBLK, OUP], bf, tag=f"k_{i}_{p}", name=f"k_{i}_{p}") for i in range(2)]
                for p in range(2)
            ]
            scratch = cpool.tile([C, HW], bf, tag="scratch")

            for s in range(BS):
                par = s % 2
                xt = xpool.tile([C + 1, HW], f32, tag="xt")
                nc.sync.dma_start(xt[:], xaug_d[s])

                # ---- routing ----
                nc.vector.tensor_reduce(
                    pooled[0:C, s : s + 1],
                    xt[0:C, :],
                    mybir.AxisListType.X,
                    OP.add,
                )
                ps_rt = rt_ps.tile([1, E], f32, tag="ps_rt")
                nc.tensor.matmul(
                    ps_rt[:], pooled[:, s : s + 1], wraug[:], start=True, stop=True
                )
                sig = xpool.tile([1, E], f32, tag="sig")
                nc.scalar.activation(sig[:], ps_rt[:], AF.Sigmoid)
                ps_r = rt_ps.tile([128, E], f32, tag="ps_r")
                nc.tensor.matmul(ps_r[:], ones128[:], sig[:], start=True, stop=True)
                rsb = xpool.tile([128, E], f32, tag="rsb")
                nc.scalar.copy(rsb[:], ps_r[:])
                # ---- k-mix on GPSIMD: k[c, b, o] = sum_e r_e * we[c, b, e, o]
                ka, kb = kt[par]
                nc.vector.tensor_scalar(
                    ka[:], wearr[:, :, 0, :], rsb[:, 0:1], None, OP.mult
                )
                cur, nxt = ka, kb
                for e in range(1, E):
                    nc.vector.scalar_tensor_tensor(
                        nxt[:], wearr[:, :, e, :], rsb[:, e : e + 1], cur[:],
                        OP.mult, OP.add,
                    )
                    cur, nxt = nxt, cur
                kfin = cur  # after 7 STT: ends in kb... (E-1 odd -> kb)

                # ---- expand conv + ReLU evac into padded h1 ----
                for b in range(NBLK):
                    hv = h1t[b][par].rearrange("p (h w) -> p h w", h=HP, w=WP)
                    for j in range(NCH):
                        ps = pe_ps.tile([128, CH], f32, tag="ps_e")
                        nc.tensor.matmul(
                            ps[:],
                            w1aug[:, b * 128 : (b + 1) * 128],
                            xt[:, j * CH : (j + 1) * CH],
                            start=True,
                            stop=True,
                        )
                        nc.scalar.activation(
                            hv[:, 2 + 8 * j : 10 + 8 * j, 1:57],
                            ps.rearrange("p (h w) -> p h w", h=8, w=56)[:],
                            AF.Relu,
                        )

                # ---- depthwise 3x3 ----
                for b in range(NBLK):
                    h1 = h1t[b][par]
                    if (s, b) in dw_pe_groups:
                        # PE path: 9 diag matmuls accumulating in PSUM per chunk
                        hv = h1.rearrange("p (h w) -> p h w", h=HP, w=WP)
                        for j in range(NCH):
                            ps = dw_ps.tile([128, CH], f32, tag="ps_dw")
                            first = True
                            for dh in (-1, 0, 1):
                                for dw_ in (-1, 0, 1):
                                    tap = (dh + 1) * 3 + (dw_ + 1)
                                    nc.tensor.matmul(
                                        ps.rearrange("p (h w) -> p h w", h=8, w=56)[:],
                                        dgm[:, b, tap, :],
                                        hv[
                                            :,
                                            2 + 8 * j + dh : 10 + 8 * j + dh,
                                            1 + dw_ : 57 + dw_,
                                        ],
                                        start=first,
                                        stop=(tap == 8),
                                    )
                                    first = False
                            nc.scalar.activation(
                                h2t[b][par][:, j * CH : (j + 1) * CH],
                                ps[:],
                                AF.Relu,
                                bias=bias2[:, b : b + 1],
                            )
                    else:
                        # DVE path: shifted copy + 9 STT MACs (all 2x-aligned)
                        sh = shpool.tile([128, FLAT], bf, tag="sh")
                        nc.sync.dma_start(sh[:, 0 : FLAT - 1], h1[:, 1:FLAT])
                        aa, ab = acct
                        first = True
                        cur_acc = None
                        for dh in (-1, 0, 1):
                            for dw_ in (-1, 0, 1):
                                tap = (dh + 1) * 3 + (dw_ + 1)
                                d = WP * dh + dw_
                                wtap = wd9[:, b, tap : tap + 1]
                                if d % 2 == 0:
                                    opnd = h1[:, REG0 + d : REG0 + d + REGN]
                                else:
                                    opnd = sh[:, REG0 + d - 1 : REG0 + d - 1 + REGN]
                                if first:
                                    nc.vector.tensor_scalar(
                                        aa[:, REG0 : REG0 + REGN],
                                        opnd, wtap, None, OP.mult,
                                    )
                                    cur_acc, nxt_acc = aa, ab
                                    first = False
                                else:
                                    nc.vector.scalar_tensor_tensor(
                                        nxt_acc[:, REG0 : REG0 + REGN],
                                        opnd, wtap,
                                        cur_acc[:, REG0 : REG0 + REGN],
                                        OP.mult, OP.add,
                                    )
                                    cur_acc, nxt_acc = nxt_acc, cur_acc
                        av = cur_acc.rearrange("p (h w) -> p h w", h=HP, w=WP)
                        nc.scalar.activation(
                            h2t[b][par].rearrange("p (h w) -> p h w", h=H, w=W)[:],
                            av[:, 2:58, 1:57],
                            AF.Relu,
                            bias=bias2[:, b : b + 1],
                        )

                # ---- condconv + residual ----
                ot = opool.tile([OUP, HW], f32, tag="ot")
                for j in range(NCH):
                    ps = cc_ps.tile([OUP, CH], f32, tag="ps_c")
                    for b in range(NBLK):
                        nc.tensor.matmul(
                            ps[:],
                            kfin[:, b, :],
                            h2t[b][par][:, j * CH : (j + 1) * CH],
                            start=(b == 0),
                            stop=False,
                        )
                    nc.tensor.matmul(
                        ps[:],
                        resid[:],
                        xt[:, j * CH : (j + 1) * CH],
                        start=False,
                        stop=True,
                    )
                    nc.scalar.copy(ot[:, j * CH : (j + 1) * CH], ps[:])
                nc.sync.dma_start(y_d[s], ot[:])

    nc.compile()
    return nc


def _prep(inputs):
    x = np.ascontiguousarray(inputs["x"].reshape(B, C, HW), dtype=np.float32)
    xaug = np.empty((B, C + 1, HW), np.float32)
    xaug[:, :C, :] = x
    xaug[:, C, :] = 1.0

    def bnfold(g, b, m, v):
        s = np.asarray(g, np.float32) / np.sqrt(np.asarray(v, np.float32) + EPS)
        return s, np.asarray(b, np.float32) - np.asarray(m, np.float32) * s

    s1, b1 = bnfold(inputs["g1"], inputs["b1"], inputs["m1"], inputs["v1"])
    s2, b2 = bnfold(inputs["g2"], inputs["b2"], inputs["m2"], inputs["v2"])
    s3, b3 = bnfold(inputs["g3"], inputs["b3"], inputs["m3"], inputs["v3"])
    w1 = np.asarray(inputs["w1"], np.float32)
    wd = np.asarray(inputs["wd"], np.float32)
    we = np.asarray(inputs["we"], np.float32)
    wr = np.asarray(inputs["wr"], np.float32)
    br = np.asarray(inputs["br"], np.float32)

    w1aug = np.zeros((C + 1, HID), np.float32)
    w1aug[:C, :] = (w1 * s1[:, None]).T
    w1aug[C, :] = b1
    wraug = np.zeros((C + 1, E), np.float32)
    wraug[:C, :] = wr.T / HW
    wraug[C, :] = br
    wd9 = (wd[:, 0, :, :] * s2[:, None, None]).reshape(HID, 9)
    wd9r = np.ascontiguousarray(wd9.reshape(NBLK, 128, 9).transpose(1, 0, 2))
    we_s = we * s3[None, :, None]                      # [E, OUP, HID]
    wearr = np.ascontiguousarray(
        np.transpose(we_s, (2, 0, 1)).reshape(NBLK, 128, E, OUP).transpose(1, 0, 2, 3)
    ).astype(bf16)
    # wearr[p, b, e, o] = we_s[e, o, 128*b + p]
    wearr = np.ascontiguousarray(
        np.transpose(we_s, (2, 0, 1))  # [c, e, o]
        .reshape(NBLK, 128, E, OUP)    # [b, p, e, o]
        .transpose(1, 0, 2, 3)         # [p, b, e, o]
    ).astype(bf16)
    resid = np.zeros((C + 1, OUP), np.float32)
    resid[:C, :] = np.eye(C, dtype=np.float32)
    resid[C, :] = b3
    # diag matrices for PE-mode DW: dgm[p, b, tap, q] = wd9[128b+p, tap] if p==q
    dgm = np.zeros((128, NBLK, 9, 128), np.float32)
    idx = np.arange(128)
    for b in range(NBLK):
        for t in range(9):
            dgm[idx, b, t, idx] = wd9[b * 128 + idx, t]
    dgm = dgm.astype(bf16)
    shared = dict(
        w1aug=w1aug, wraug=wraug, wd9=wd9r, wd9b=wd9r.astype(bf16), wearr=wearr,
        resid=resid, bias2=np.ascontiguousarray(b2.reshape(NBLK, 128).T), dgm=dgm,
    )
    return xaug, shared


DW_PE_GROUPS = frozenset({(s, b) for s in range(BS) for b in (0, 1)})


def kernel(**inputs):
    from concourse.bass_utils import run_bass_kernel_spmd

    key = ("k", DW_PE_GROUPS)
    if key not in _CACHE:
        _CACHE[key] = _build(DW_PE_GROUPS)
    nc = _CACHE[key]

    xaug, shared = _prep(inputs)
    if not DW_PE_GROUPS:
        shared = {k: v for k, v in shared.items() if k != "dgm"}
    in_maps = [
        dict(shared, xaug=np.ascontiguousarray(xaug[c * BS : (c + 1) * BS]))
        for c in range(NCORES)
    ]
    res = run_bass_kernel_spmd(nc, in_maps, core_ids=list(range(NCORES)))
    y = np.concatenate([r["y"] for r in res.results], axis=0)
    return y.reshape(B, OUP, H, W).astype(np.float32)
